# revision 73
# baseline (speedup 1.0000x reference)
"""Trainium2 Bass kernel for nn_Attention_23218593202595.

reference:
    hidden = concat([static, dynamic, broadcast(decoder)], axis=1)   # [B, 3H, S]
    u      = tanh(einsum('hk,bks->bhs', W[0], hidden))               # [B, H, S]
    scores = einsum('h,bhs->bs', v[0,0], u)[:, None, :]              # [B, 1, S]
    out    = softmax(scores, axis=2)

B=256, H=256, S=512.  Pure data parallel over 8 NeuronCores: core i owns
batches [32i, 32i+32).  W/v/decoder-projection are tiny and replicated.

Per core, per batch b (q = b%4, one PSUM score bank per 4-batch group):
    psum_u[mc]          = sum_kc Wt[kc, mc]^T @ x[kc]   (x = [static;dynamic], bf16)
    u[mc]               = tanh(psum_u[mc] + c[:, b])    (ScalarE, c = W_dec @ dec)
    sbank[32q:32q+32]  += (v ox 1_32)[mc]^T @ u[mc]     (M=32 matmul -> 32 equal rows,
                                                         pipelined 1 batch behind)
After each 4-batch group the softmax runs directly on the psum bank
(reduce-max -> exp w/ fused sum -> reciprocal -> scale) and rows
{0,32,64,96} are DMAed to the output.

All inputs are converted to bf16 and pre-swizzled on the host into
partition-major layouts so every DMA lands as 128 contiguous runs.
"""
import sys

if "/opt/trn_rl_repo" not in sys.path:
    sys.path.insert(0, "/opt/trn_rl_repo")

import numpy as np
import ml_dtypes

B, H, S = 256, 256, 512
N_CORES = 8
NB = B // N_CORES                    # batches per core
GROUPS = [1, 1, 1, 1] + [2] * 14     # batches per dma group (sum = NB)

_cache = {}


def _build():
    import concourse.bass as bass
    import concourse.mybir as mybir
    import concourse.tile as tile
    from concourse import bacc

    f32 = mybir.dt.float32
    bf16 = mybir.dt.bfloat16
    AF = mybir.ActivationFunctionType

    nc = bacc.Bacc("TRN2", target_bir_lowering=False, debug=False,
                   num_devices=N_CORES)

    # all host-preswizzled, partition-major.  xin interleaves static+dynamic
    # per batch as [b, tensor, c, s] so one DMA covers a whole batch group;
    # consts packs wt | dect | vv into a single transfer.
    CW = 6 * H + 2 * NB + 2 * 32
    xin_ext = nc.declare_dram_parameter("xin", [128, NB * 4 * S], bf16, isOutput=False)
    cst_ext = nc.declare_dram_parameter("consts", [128, CW], bf16, isOutput=False)
    out_ext = nc.declare_dram_parameter("out", [NB, S], f32, isOutput=True)

    with tile.TileContext(nc) as tc:
        with (
            tc.tile_pool(name="const", bufs=1) as cpool,
            tc.tile_pool(name="inp", bufs=4) as ipool,
            tc.tile_pool(name="upool", bufs=17) as upool,
            tc.tile_pool(name="smpool", bufs=2) as smpool,
            tc.tile_pool(name="ps_u", bufs=4, space=bass.MemorySpace.PSUM) as ps_u,
            tc.tile_pool(name="ps_s", bufs=4, space=bass.MemorySpace.PSUM) as ps_s,
        ):
            # ---- constants: one DMA, first on the sync queue ----
            cst_sb = cpool.tile([128, CW], bf16)
            nc.sync.dma_start(out=cst_sb[:], in_=cst_ext.ap())
            wt_sb = cst_sb[:, :6 * H] \
                .rearrange("p (c m) -> p c m", m=H)            # [p, kc, m]
            de_sb = cst_sb[:, 6 * H:6 * H + 2 * NB] \
                .rearrange("p (c b) -> p c b", b=NB)           # [p, c, b]
            vv_sb = cst_sb[:, 6 * H + 2 * NB:] \
                .rearrange("p (c m) -> p c m", m=32)           # [p, c, m] v x32

            # PE warmup while DMAs land: dense N=512 matmuls on a zeroed
            # tile so the HAM clock gate opens before the real stream starts
            warm = cpool.tile([128, S], bf16)
            nc.gpsimd.memset(warm[:], 0.0)
            wpsum = ps_s.tile([64, S], f32, tag="sps")
            for _ in range(10):
                nc.tensor.matmul(wpsum[:], warm[:, :64], warm[:])

            cbias = cpool.tile([128, 2, NB], f32)

            def emit_cmms():
                # c = W_dec @ dec^T  -> [H, NB] f32, kept as tanh bias
                for mc in range(2):
                    pc = ps_s.tile([128, NB], f32, tag="sps")
                    for kc in range(2):
                        nc.tensor.matmul(
                            pc[:],
                            wt_sb[:, 4 + kc, mc * 128:(mc + 1) * 128],
                            de_sb[:, kc, :],
                            start=(kc == 0), stop=(kc == 1),
                        )
                    nc.vector.tensor_copy(cbias[:, mc, :], pc[:])

            def score_mms(sbank, q, u_list):
                # M=32 matmul pair -> score row of one batch replicated
                # across the bank's q-th 32-partition slice
                for mc in range(2):
                    nc.tensor.matmul(sbank[32 * q:32 * (q + 1), :],
                                     vv_sb[:, mc, :], u_list[mc][:],
                                     start=(mc == 0), stop=(mc == 1),
                                     tile_position=(0, 32 * q))

            def group_softmax(g, sbank):
                # softmax for batches 4g..4g+3 straight off the psum bank.
                # no max-subtraction: |score| <= ||v||_1 ~ 10, exp stays
                # comfortably inside f32 range, matching the reference to
                # rounding error.
                prob = smpool.tile([128, S], f32, tag="pr")
                sums = smpool.tile([128, 1], f32, tag="sm")
                nc.scalar.activation(prob[:], sbank[:], AF.Exp,
                                     accum_out=sums[:])
                recip = smpool.tile([128, 1], f32, tag="rc")
                nc.vector.reciprocal(recip[:], sums[:])
                outp = smpool.tile([128, S], f32, tag="op")
                nc.vector.tensor_scalar_mul(outp[:], prob[:], recip[:])
                # early groups ride the idle gpsimd SWDGE so the sync queue
                # keeps feeding input; the last group stays on sync (SWDGE's
                # ~2us completion latency would land in the kernel tail)
                dma_eng = nc.sync if g == NB // 4 - 1 else nc.gpsimd
                dma_eng.dma_start(
                    out=out_ext.ap()[4 * g:4 * (g + 1), :],
                    in_=outp[:].rearrange("(q r) s -> q r s", r=32)[:, 0, :])

            def emit_group_scores(g, items):
                sbank = ps_s.tile([128, S], f32, tag="sps")
                for q, (b, u_list) in enumerate(items):
                    score_mms(sbank, q, u_list)
                group_softmax(g, sbank)

            # ---- main loop over batches ----
            state = {}
            pending = []
            b0 = 0

            # c-matmuls need only consts (which lands first) — run them
            # before batch 0 so they fill the consts->x0 arrival gap
            emit_cmms()

            for gi, gsz in enumerate(GROUPS):
                x_t = ipool.tile([128, gsz, 2, 2, S], bf16, tag="xt")
                nc.sync.dma_start(
                    out=x_t[:],
                    in_=xin_ext.ap()[:, b0 * 4 * S:(b0 + gsz) * 4 * S])

                for j in range(gsz):
                    b = b0 + j
                    psu_list = []
                    for mc in range(2):
                        psu = ps_u.tile([128, S], f32, tag="ups")
                        for kc in range(2):
                            nc.tensor.matmul(
                                psu[:],
                                wt_sb[:, kc, mc * 128:(mc + 1) * 128],
                                x_t[:, j, 0, kc, :],
                                start=(kc == 0), stop=False,
                            )
                        for kc in range(2):
                            nc.tensor.matmul(
                                psu[:],
                                wt_sb[:, 2 + kc, mc * 128:(mc + 1) * 128],
                                x_t[:, j, 1, kc, :],
                                start=False, stop=(kc == 1),
                            )
                        psu_list.append(psu)
                    u_list = []
                    for mc in range(2):
                        u_bf = upool.tile([128, S], bf16, tag="u")
                        nc.scalar.activation(u_bf[:], psu_list[mc][:], AF.Tanh,
                                             bias=cbias[:, mc, b:b + 1])
                        u_list.append(u_bf)
                    # group score matmuls run 2+ batches behind the mains
                    pending.append((b, u_list))
                    if len(pending) >= 5:
                        g = pending[0][0] // 4
                        emit_group_scores(g, pending[:4])
                        pending = pending[4:]
                b0 += gsz
            while pending:
                g = pending[0][0] // 4
                emit_group_scores(g, pending[:4])
                pending = pending[4:]

    nc.compile()
    return nc


def _get_nc():
    if "nc" not in _cache:
        _cache["nc"] = _build()
    return _cache["nc"]


def make_in_maps(static_hidden, dynamic_hidden, decoder_hidden, W, v):
    bf = ml_dtypes.bfloat16
    # W[0] is [H, 3H]; wt[p, kc*H + m] = W[0][m, kc*128 + p]
    wt = W[0].T.astype(bf).reshape(6, 128, H).transpose(1, 0, 2).reshape(128, 6 * H)
    # v replicated 32x: vv[p, c*32 + m] = v[c*128 + p]
    vv = np.repeat(v[0, 0].astype(bf).reshape(2, 128).T[:, :, None], 32,
                   axis=2).reshape(128, 64)

    sh = static_hidden.astype(bf).reshape(N_CORES, NB, 2, 128, S)
    dh = dynamic_hidden.astype(bf).reshape(N_CORES, NB, 2, 128, S)
    # xin[p, b, t, c, s]; h = c*128 + p
    xin_all = np.stack([sh, dh], axis=2).transpose(0, 4, 1, 2, 3, 5) \
        .reshape(N_CORES, 128, NB * 4 * S)

    in_maps = []
    for i in range(N_CORES):
        sl = slice(i * NB, (i + 1) * NB)
        dect = decoder_hidden[sl].T.astype(bf).reshape(2, 128, NB) \
            .transpose(1, 0, 2).reshape(128, 2 * NB)
        consts = np.concatenate([wt, dect, vv], axis=1)
        in_maps.append({
            "xin": np.ascontiguousarray(xin_all[i]),
            "consts": np.ascontiguousarray(consts),
        })
    return in_maps


def kernel(static_hidden, dynamic_hidden, decoder_hidden, W, v):
    from concourse.bass_utils import run_bass_kernel_spmd

    static_hidden = np.asarray(static_hidden, dtype=np.float32)
    dynamic_hidden = np.asarray(dynamic_hidden, dtype=np.float32)
    decoder_hidden = np.asarray(decoder_hidden, dtype=np.float32)
    W = np.asarray(W, dtype=np.float32)
    v = np.asarray(v, dtype=np.float32)
    nc = _get_nc()
    in_maps = make_in_maps(static_hidden, dynamic_hidden, decoder_hidden, W, v)
    res = run_bass_kernel_spmd(nc, in_maps, list(range(N_CORES)))
    out = np.concatenate([res.results[i]["out"] for i in range(N_CORES)], axis=0)
    return out[:, None, :].astype(np.float32)


# revision 74
# speedup vs baseline: 1.0105x; 1.0105x over previous
"""Trainium2 Bass kernel for nn_Attention_23218593202595.

reference:
    hidden = concat([static, dynamic, broadcast(decoder)], axis=1)   # [B, 3H, S]
    u      = tanh(einsum('hk,bks->bhs', W[0], hidden))               # [B, H, S]
    scores = einsum('h,bhs->bs', v[0,0], u)[:, None, :]              # [B, 1, S]
    out    = softmax(scores, axis=2)

B=256, H=256, S=512.  Pure data parallel over 8 NeuronCores: core i owns
batches [32i, 32i+32).  W/v/decoder-projection are tiny and replicated.

Per core, per batch b (q = b%4, one PSUM score bank per 4-batch group):
    psum_u[mc]          = sum_kc Wt[kc, mc]^T @ x[kc]   (x = [static;dynamic], bf16)
    u[mc]               = tanh(psum_u[mc] + c[:, b])    (ScalarE, c = W_dec @ dec)
    sbank[32q:32q+32]  += (v ox 1_32)[mc]^T @ u[mc]     (M=32 matmul -> 32 equal rows,
                                                         pipelined 1 batch behind)
After each 4-batch group the softmax runs directly on the psum bank
(reduce-max -> exp w/ fused sum -> reciprocal -> scale) and rows
{0,32,64,96} are DMAed to the output.

All inputs are converted to bf16 and pre-swizzled on the host into
partition-major layouts so every DMA lands as 128 contiguous runs.
"""
import sys

if "/opt/trn_rl_repo" not in sys.path:
    sys.path.insert(0, "/opt/trn_rl_repo")

import numpy as np
import ml_dtypes

B, H, S = 256, 256, 512
N_CORES = 8
NB = B // N_CORES                    # batches per core
GROUPS = [1, 1] + [2] * 15           # batches per dma group (sum = NB)

_cache = {}


def _build():
    import concourse.bass as bass
    import concourse.mybir as mybir
    import concourse.tile as tile
    from concourse import bacc

    f32 = mybir.dt.float32
    bf16 = mybir.dt.bfloat16
    AF = mybir.ActivationFunctionType

    nc = bacc.Bacc("TRN2", target_bir_lowering=False, debug=False,
                   num_devices=N_CORES)

    # all host-preswizzled, partition-major.  xin interleaves static+dynamic
    # per batch as [b, tensor, c, s] so one DMA covers a whole batch group;
    # consts packs wt | dect | vv into a single transfer.
    CW = 6 * H + 2 * NB + 2 * 32
    xin_ext = nc.declare_dram_parameter("xin", [128, NB * 4 * S], bf16, isOutput=False)
    cst_ext = nc.declare_dram_parameter("consts", [128, CW], bf16, isOutput=False)
    out_ext = nc.declare_dram_parameter("out", [NB, S], f32, isOutput=True)

    with tile.TileContext(nc) as tc:
        with (
            tc.tile_pool(name="const", bufs=1) as cpool,
            tc.tile_pool(name="inp", bufs=4) as ipool,
            tc.tile_pool(name="upool", bufs=17) as upool,
            tc.tile_pool(name="smpool", bufs=2) as smpool,
            tc.tile_pool(name="ps_u", bufs=5, space=bass.MemorySpace.PSUM) as ps_u,
            tc.tile_pool(name="ps_s", bufs=3, space=bass.MemorySpace.PSUM) as ps_s,
        ):
            # ---- constants: one DMA, first on the sync queue ----
            cst_sb = cpool.tile([128, CW], bf16)
            nc.sync.dma_start(out=cst_sb[:], in_=cst_ext.ap())
            wt_sb = cst_sb[:, :6 * H] \
                .rearrange("p (c m) -> p c m", m=H)            # [p, kc, m]
            de_sb = cst_sb[:, 6 * H:6 * H + 2 * NB] \
                .rearrange("p (c b) -> p c b", b=NB)           # [p, c, b]
            vv_sb = cst_sb[:, 6 * H + 2 * NB:] \
                .rearrange("p (c m) -> p c m", m=32)           # [p, c, m] v x32

            # PE warmup while DMAs land: dense N=512 matmuls on a zeroed
            # tile so the HAM clock gate opens before the real stream starts
            warm = cpool.tile([128, S], bf16)
            nc.gpsimd.memset(warm[:], 0.0)
            wpsum = ps_s.tile([64, S], f32, tag="sps")
            for _ in range(10):
                nc.tensor.matmul(wpsum[:], warm[:, :64], warm[:])

            cbias = cpool.tile([128, 2, NB], f32)

            def emit_cmms():
                # c = W_dec @ dec^T  -> [H, NB] f32, kept as tanh bias
                for mc in range(2):
                    pc = ps_s.tile([128, NB], f32, tag="sps")
                    for kc in range(2):
                        nc.tensor.matmul(
                            pc[:],
                            wt_sb[:, 4 + kc, mc * 128:(mc + 1) * 128],
                            de_sb[:, kc, :],
                            start=(kc == 0), stop=(kc == 1),
                        )
                    nc.vector.tensor_copy(cbias[:, mc, :], pc[:])

            def score_mms(sbank, q, u_list):
                # M=32 matmul pair -> score row of one batch replicated
                # across the bank's q-th 32-partition slice
                for mc in range(2):
                    nc.tensor.matmul(sbank[32 * q:32 * (q + 1), :],
                                     vv_sb[:, mc, :], u_list[mc][:],
                                     start=(mc == 0), stop=(mc == 1),
                                     tile_position=(0, 32 * q))

            def group_softmax(g, sbank):
                # softmax for batches 4g..4g+3 straight off the psum bank.
                # no max-subtraction: |score| <= ||v||_1 ~ 10, exp stays
                # comfortably inside f32 range, matching the reference to
                # rounding error.
                prob = smpool.tile([128, S], f32, tag="pr")
                sums = smpool.tile([128, 1], f32, tag="sm")
                nc.scalar.activation(prob[:], sbank[:], AF.Exp,
                                     accum_out=sums[:])
                recip = smpool.tile([128, 1], f32, tag="rc")
                nc.vector.reciprocal(recip[:], sums[:])
                outp = smpool.tile([128, S], f32, tag="op")
                nc.vector.tensor_scalar_mul(outp[:], prob[:], recip[:])
                # early groups ride the idle gpsimd SWDGE so the sync queue
                # keeps feeding input; the last group stays on sync (SWDGE's
                # ~2us completion latency would land in the kernel tail)
                dma_eng = nc.sync if g == NB // 4 - 1 else nc.gpsimd
                dma_eng.dma_start(
                    out=out_ext.ap()[4 * g:4 * (g + 1), :],
                    in_=outp[:].rearrange("(q r) s -> q r s", r=32)[:, 0, :])

            def emit_group_scores(g, items):
                sbank = ps_s.tile([128, S], f32, tag="sps")
                for q, (b, u_list) in enumerate(items):
                    score_mms(sbank, q, u_list)
                group_softmax(g, sbank)

            # ---- main loop over batches ----
            state = {}
            pending = []
            b0 = 0

            # c-matmuls need only consts (which lands first) — run them
            # before batch 0 so they fill the consts->x0 arrival gap
            emit_cmms()

            for gi, gsz in enumerate(GROUPS):
                x_t = ipool.tile([128, gsz, 2, 2, S], bf16, tag="xt")
                nc.sync.dma_start(
                    out=x_t[:],
                    in_=xin_ext.ap()[:, b0 * 4 * S:(b0 + gsz) * 4 * S])

                for j in range(gsz):
                    b = b0 + j
                    psu_list = []
                    for mc in range(2):
                        psu = ps_u.tile([128, S], f32, tag="ups")
                        for kc in range(2):
                            nc.tensor.matmul(
                                psu[:],
                                wt_sb[:, kc, mc * 128:(mc + 1) * 128],
                                x_t[:, j, 0, kc, :],
                                start=(kc == 0), stop=False,
                            )
                        for kc in range(2):
                            nc.tensor.matmul(
                                psu[:],
                                wt_sb[:, 2 + kc, mc * 128:(mc + 1) * 128],
                                x_t[:, j, 1, kc, :],
                                start=False, stop=(kc == 1),
                            )
                        psu_list.append(psu)
                    u_list = []
                    for mc in range(2):
                        u_bf = upool.tile([128, S], bf16, tag="u")
                        nc.scalar.activation(u_bf[:], psu_list[mc][:], AF.Tanh,
                                             bias=cbias[:, mc, b:b + 1])
                        u_list.append(u_bf)
                    # group score matmuls run 2+ batches behind the mains
                    pending.append((b, u_list))
                    if len(pending) >= 5:
                        g = pending[0][0] // 4
                        emit_group_scores(g, pending[:4])
                        pending = pending[4:]
                b0 += gsz
            while pending:
                g = pending[0][0] // 4
                emit_group_scores(g, pending[:4])
                pending = pending[4:]

    nc.compile()
    return nc


def _get_nc():
    if "nc" not in _cache:
        _cache["nc"] = _build()
    return _cache["nc"]


def make_in_maps(static_hidden, dynamic_hidden, decoder_hidden, W, v):
    bf = ml_dtypes.bfloat16
    # W[0] is [H, 3H]; wt[p, kc*H + m] = W[0][m, kc*128 + p]
    wt = W[0].T.astype(bf).reshape(6, 128, H).transpose(1, 0, 2).reshape(128, 6 * H)
    # v replicated 32x: vv[p, c*32 + m] = v[c*128 + p]
    vv = np.repeat(v[0, 0].astype(bf).reshape(2, 128).T[:, :, None], 32,
                   axis=2).reshape(128, 64)

    sh = static_hidden.astype(bf).reshape(N_CORES, NB, 2, 128, S)
    dh = dynamic_hidden.astype(bf).reshape(N_CORES, NB, 2, 128, S)
    # xin[p, b, t, c, s]; h = c*128 + p
    xin_all = np.stack([sh, dh], axis=2).transpose(0, 4, 1, 2, 3, 5) \
        .reshape(N_CORES, 128, NB * 4 * S)

    in_maps = []
    for i in range(N_CORES):
        sl = slice(i * NB, (i + 1) * NB)
        dect = decoder_hidden[sl].T.astype(bf).reshape(2, 128, NB) \
            .transpose(1, 0, 2).reshape(128, 2 * NB)
        consts = np.concatenate([wt, dect, vv], axis=1)
        in_maps.append({
            "xin": np.ascontiguousarray(xin_all[i]),
            "consts": np.ascontiguousarray(consts),
        })
    return in_maps


def kernel(static_hidden, dynamic_hidden, decoder_hidden, W, v):
    from concourse.bass_utils import run_bass_kernel_spmd

    static_hidden = np.asarray(static_hidden, dtype=np.float32)
    dynamic_hidden = np.asarray(dynamic_hidden, dtype=np.float32)
    decoder_hidden = np.asarray(decoder_hidden, dtype=np.float32)
    W = np.asarray(W, dtype=np.float32)
    v = np.asarray(v, dtype=np.float32)
    nc = _get_nc()
    in_maps = make_in_maps(static_hidden, dynamic_hidden, decoder_hidden, W, v)
    res = run_bass_kernel_spmd(nc, in_maps, list(range(N_CORES)))
    out = np.concatenate([res.results[i]["out"] for i in range(N_CORES)], axis=0)
    return out[:, None, :].astype(np.float32)


# revision 75
# speedup vs baseline: 1.0105x; 1.0000x over previous
"""Trainium2 Bass kernel for nn_Attention_23218593202595.

reference:
    hidden = concat([static, dynamic, broadcast(decoder)], axis=1)   # [B, 3H, S]
    u      = tanh(einsum('hk,bks->bhs', W[0], hidden))               # [B, H, S]
    scores = einsum('h,bhs->bs', v[0,0], u)[:, None, :]              # [B, 1, S]
    out    = softmax(scores, axis=2)

B=256, H=256, S=512.  Pure data parallel over 8 NeuronCores: core i owns
batches [32i, 32i+32).  W/v/decoder-projection are tiny and replicated.

Per core, per batch b (q = b%4, one PSUM score bank per 4-batch group):
    psum_u[mc]          = sum_kc Wt[kc, mc]^T @ x[kc]   (x = [static;dynamic], bf16)
    u[mc]               = tanh(psum_u[mc] + c[:, b])    (ScalarE, c = W_dec @ dec)
    sbank[32q:32q+32]  += (v ox 1_32)[mc]^T @ u[mc]     (M=32 matmul -> 32 equal rows,
                                                         pipelined 1 batch behind)
After each 4-batch group the softmax runs directly on the psum bank
(reduce-max -> exp w/ fused sum -> reciprocal -> scale) and rows
{0,32,64,96} are DMAed to the output.

All inputs are converted to bf16 and pre-swizzled on the host into
partition-major layouts so every DMA lands as 128 contiguous runs.
"""
import sys

if "/opt/trn_rl_repo" not in sys.path:
    sys.path.insert(0, "/opt/trn_rl_repo")

import numpy as np
import ml_dtypes

B, H, S = 256, 256, 512
N_CORES = 8
NB = B // N_CORES                    # batches per core
GROUPS = [1, 1] + [2] * 15           # batches per dma group (sum = NB)

_cache = {}


def _build():
    import concourse.bass as bass
    import concourse.mybir as mybir
    import concourse.tile as tile
    from concourse import bacc

    f32 = mybir.dt.float32
    bf16 = mybir.dt.bfloat16
    AF = mybir.ActivationFunctionType

    nc = bacc.Bacc("TRN2", target_bir_lowering=False, debug=False,
                   num_devices=N_CORES)

    # all host-preswizzled, partition-major.  xin interleaves static+dynamic
    # per batch as [b, tensor, c, s] so one DMA covers a whole batch group;
    # consts packs wt | dect | vv into a single transfer.
    CW = 6 * H + 2 * NB + 2 * 32
    xin_ext = nc.declare_dram_parameter("xin", [128, NB * 4 * S], bf16, isOutput=False)
    cst_ext = nc.declare_dram_parameter("consts", [128, CW], bf16, isOutput=False)
    out_ext = nc.declare_dram_parameter("out", [NB, S], f32, isOutput=True)

    with tile.TileContext(nc) as tc:
        with (
            tc.tile_pool(name="const", bufs=1) as cpool,
            tc.tile_pool(name="inp", bufs=4) as ipool,
            tc.tile_pool(name="upool", bufs=17) as upool,
            tc.tile_pool(name="smpool", bufs=2) as smpool,
            tc.tile_pool(name="ps_u", bufs=4, space=bass.MemorySpace.PSUM) as ps_u,
            tc.tile_pool(name="ps_s", bufs=4, space=bass.MemorySpace.PSUM) as ps_s,
        ):
            # ---- constants: one DMA, first on the sync queue ----
            cst_sb = cpool.tile([128, CW], bf16)
            nc.sync.dma_start(out=cst_sb[:], in_=cst_ext.ap())
            wt_sb = cst_sb[:, :6 * H] \
                .rearrange("p (c m) -> p c m", m=H)            # [p, kc, m]
            de_sb = cst_sb[:, 6 * H:6 * H + 2 * NB] \
                .rearrange("p (c b) -> p c b", b=NB)           # [p, c, b]
            vv_sb = cst_sb[:, 6 * H + 2 * NB:] \
                .rearrange("p (c m) -> p c m", m=32)           # [p, c, m] v x32

            # PE warmup while DMAs land: dense N=512 matmuls on a zeroed
            # tile so the HAM clock gate opens before the real stream starts
            warm = cpool.tile([128, S], bf16)
            nc.gpsimd.memset(warm[:], 0.0)
            wpsum = ps_s.tile([64, S], f32, tag="sps")
            for _ in range(10):
                nc.tensor.matmul(wpsum[:], warm[:, :64], warm[:])

            cbias = cpool.tile([128, 2, NB], f32)

            def emit_cmms():
                # c = W_dec @ dec^T  -> [H, NB] f32, kept as tanh bias
                for mc in range(2):
                    pc = ps_s.tile([128, NB], f32, tag="sps")
                    for kc in range(2):
                        nc.tensor.matmul(
                            pc[:],
                            wt_sb[:, 4 + kc, mc * 128:(mc + 1) * 128],
                            de_sb[:, kc, :],
                            start=(kc == 0), stop=(kc == 1),
                        )
                    nc.vector.tensor_copy(cbias[:, mc, :], pc[:])

            def score_mms(sbank, q, u_list):
                # M=32 matmul pair -> score row of one batch replicated
                # across the bank's q-th 32-partition slice
                for mc in range(2):
                    nc.tensor.matmul(sbank[32 * q:32 * (q + 1), :],
                                     vv_sb[:, mc, :], u_list[mc][:],
                                     start=(mc == 0), stop=(mc == 1),
                                     tile_position=(0, 32 * q))

            def group_softmax(g, sbank):
                # softmax for batches 4g..4g+3 straight off the psum bank.
                # no max-subtraction: |score| <= ||v||_1 ~ 10, exp stays
                # comfortably inside f32 range, matching the reference to
                # rounding error.
                prob = smpool.tile([128, S], f32, tag="pr")
                sums = smpool.tile([128, 1], f32, tag="sm")
                nc.scalar.activation(prob[:], sbank[:], AF.Exp,
                                     accum_out=sums[:])
                recip = smpool.tile([128, 1], f32, tag="rc")
                nc.vector.reciprocal(recip[:], sums[:])
                outp = smpool.tile([128, S], f32, tag="op")
                nc.vector.tensor_scalar_mul(outp[:], prob[:], recip[:])
                # early groups ride the idle gpsimd SWDGE so the sync queue
                # keeps feeding input; the last group stays on sync (SWDGE's
                # ~2us completion latency would land in the kernel tail)
                dma_eng = nc.sync if g == NB // 4 - 1 else nc.gpsimd
                dma_eng.dma_start(
                    out=out_ext.ap()[4 * g:4 * (g + 1), :],
                    in_=outp[:].rearrange("(q r) s -> q r s", r=32)[:, 0, :])

            def emit_group_scores(g, items):
                sbank = ps_s.tile([128, S], f32, tag="sps")
                for q, (b, u_list) in enumerate(items):
                    score_mms(sbank, q, u_list)
                group_softmax(g, sbank)

            # ---- main loop over batches ----
            state = {}
            pending = []
            b0 = 0

            # c-matmuls need only consts (which lands first) — run them
            # before batch 0 so they fill the consts->x0 arrival gap
            emit_cmms()

            for gi, gsz in enumerate(GROUPS):
                x_t = ipool.tile([128, gsz, 2, 2, S], bf16, tag="xt")
                nc.sync.dma_start(
                    out=x_t[:],
                    in_=xin_ext.ap()[:, b0 * 4 * S:(b0 + gsz) * 4 * S])

                for j in range(gsz):
                    b = b0 + j
                    psu_list = []
                    for mc in range(2):
                        psu = ps_u.tile([128, S], f32, tag="ups")
                        for kc in range(2):
                            nc.tensor.matmul(
                                psu[:],
                                wt_sb[:, kc, mc * 128:(mc + 1) * 128],
                                x_t[:, j, 0, kc, :],
                                start=(kc == 0), stop=False,
                            )
                        for kc in range(2):
                            nc.tensor.matmul(
                                psu[:],
                                wt_sb[:, 2 + kc, mc * 128:(mc + 1) * 128],
                                x_t[:, j, 1, kc, :],
                                start=False, stop=(kc == 1),
                            )
                        psu_list.append(psu)
                    u_list = []
                    for mc in range(2):
                        u_bf = upool.tile([128, S], bf16, tag="u")
                        nc.scalar.activation(u_bf[:], psu_list[mc][:], AF.Tanh,
                                             bias=cbias[:, mc, b:b + 1])
                        u_list.append(u_bf)
                    # group score matmuls run 2+ batches behind the mains
                    pending.append((b, u_list))
                    if len(pending) >= 5:
                        g = pending[0][0] // 4
                        emit_group_scores(g, pending[:4])
                        pending = pending[4:]
                b0 += gsz
            while pending:
                g = pending[0][0] // 4
                emit_group_scores(g, pending[:4])
                pending = pending[4:]

    nc.compile()
    return nc


def _get_nc():
    if "nc" not in _cache:
        _cache["nc"] = _build()
    return _cache["nc"]


def make_in_maps(static_hidden, dynamic_hidden, decoder_hidden, W, v):
    bf = ml_dtypes.bfloat16
    # W[0] is [H, 3H]; wt[p, kc*H + m] = W[0][m, kc*128 + p]
    wt = W[0].T.astype(bf).reshape(6, 128, H).transpose(1, 0, 2).reshape(128, 6 * H)
    # v replicated 32x: vv[p, c*32 + m] = v[c*128 + p]
    vv = np.repeat(v[0, 0].astype(bf).reshape(2, 128).T[:, :, None], 32,
                   axis=2).reshape(128, 64)

    sh = static_hidden.astype(bf).reshape(N_CORES, NB, 2, 128, S)
    dh = dynamic_hidden.astype(bf).reshape(N_CORES, NB, 2, 128, S)
    # xin[p, b, t, c, s]; h = c*128 + p
    xin_all = np.stack([sh, dh], axis=2).transpose(0, 4, 1, 2, 3, 5) \
        .reshape(N_CORES, 128, NB * 4 * S)

    in_maps = []
    for i in range(N_CORES):
        sl = slice(i * NB, (i + 1) * NB)
        dect = decoder_hidden[sl].T.astype(bf).reshape(2, 128, NB) \
            .transpose(1, 0, 2).reshape(128, 2 * NB)
        consts = np.concatenate([wt, dect, vv], axis=1)
        in_maps.append({
            "xin": np.ascontiguousarray(xin_all[i]),
            "consts": np.ascontiguousarray(consts),
        })
    return in_maps


def kernel(static_hidden, dynamic_hidden, decoder_hidden, W, v):
    from concourse.bass_utils import run_bass_kernel_spmd

    static_hidden = np.asarray(static_hidden, dtype=np.float32)
    dynamic_hidden = np.asarray(dynamic_hidden, dtype=np.float32)
    decoder_hidden = np.asarray(decoder_hidden, dtype=np.float32)
    W = np.asarray(W, dtype=np.float32)
    v = np.asarray(v, dtype=np.float32)
    nc = _get_nc()
    in_maps = make_in_maps(static_hidden, dynamic_hidden, decoder_hidden, W, v)
    res = run_bass_kernel_spmd(nc, in_maps, list(range(N_CORES)))
    out = np.concatenate([res.results[i]["out"] for i in range(N_CORES)], axis=0)
    return out[:, None, :].astype(np.float32)


# revision 77
# speedup vs baseline: 1.0135x; 1.0030x over previous
"""Trainium2 Bass kernel for nn_Attention_23218593202595.

reference:
    hidden = concat([static, dynamic, broadcast(decoder)], axis=1)   # [B, 3H, S]
    u      = tanh(einsum('hk,bks->bhs', W[0], hidden))               # [B, H, S]
    scores = einsum('h,bhs->bs', v[0,0], u)[:, None, :]              # [B, 1, S]
    out    = softmax(scores, axis=2)

B=256, H=256, S=512.  Pure data parallel over 8 NeuronCores: core i owns
batches [32i, 32i+32).  W/v/decoder-projection are tiny and replicated.

Per core, per batch b (q = b%4, one PSUM score bank per 4-batch group):
    psum_u[mc]          = sum_kc Wt[kc, mc]^T @ x[kc]   (x = [static;dynamic], bf16)
    u[mc]               = tanh(psum_u[mc] + c[:, b])    (ScalarE, c = W_dec @ dec)
    sbank[32q:32q+32]  += (v ox 1_32)[mc]^T @ u[mc]     (M=32 matmul -> 32 equal rows,
                                                         pipelined 1 batch behind)
After each 4-batch group the softmax runs directly on the psum bank
(reduce-max -> exp w/ fused sum -> reciprocal -> scale) and rows
{0,32,64,96} are DMAed to the output.

All inputs are converted to bf16 and pre-swizzled on the host into
partition-major layouts so every DMA lands as 128 contiguous runs.
"""
import sys

if "/opt/trn_rl_repo" not in sys.path:
    sys.path.insert(0, "/opt/trn_rl_repo")

import numpy as np
import ml_dtypes

B, H, S = 256, 256, 512
N_CORES = 8
NB = B // N_CORES                    # batches per core
GROUPS = [1, 1] + [2] * 15           # batches per dma group (sum = NB)

_cache = {}


def _build():
    import concourse.bass as bass
    import concourse.mybir as mybir
    import concourse.tile as tile
    from concourse import bacc

    f32 = mybir.dt.float32
    bf16 = mybir.dt.bfloat16
    AF = mybir.ActivationFunctionType

    nc = bacc.Bacc("TRN2", target_bir_lowering=False, debug=False,
                   num_devices=N_CORES)

    # all host-preswizzled, partition-major.  xin interleaves static+dynamic
    # per batch as [b, tensor, c, s] so one DMA covers a whole batch group;
    # consts packs wt | dect | vv into a single transfer.
    CW = 6 * H + 2 * NB + 2 * 32
    xin_ext = nc.declare_dram_parameter("xin", [128, NB * 4 * S], bf16, isOutput=False)
    cst_ext = nc.declare_dram_parameter("consts", [128, CW], bf16, isOutput=False)
    out_ext = nc.declare_dram_parameter("out", [NB, S], f32, isOutput=True)

    with tile.TileContext(nc) as tc:
        with (
            tc.tile_pool(name="const", bufs=1) as cpool,
            tc.tile_pool(name="inp", bufs=4) as ipool,
            tc.tile_pool(name="upool", bufs=17) as upool,
            tc.tile_pool(name="smpool", bufs=2) as smpool,
            tc.tile_pool(name="ps_u", bufs=4, space=bass.MemorySpace.PSUM) as ps_u,
            tc.tile_pool(name="ps_s", bufs=4, space=bass.MemorySpace.PSUM) as ps_s,
        ):
            # ---- constants: one DMA, first on the sync queue ----
            cst_sb = cpool.tile([128, CW], bf16)
            nc.sync.dma_start(out=cst_sb[:], in_=cst_ext.ap())
            wt_sb = cst_sb[:, :6 * H] \
                .rearrange("p (c m) -> p c m", m=H)            # [p, kc, m]
            de_sb = cst_sb[:, 6 * H:6 * H + 2 * NB] \
                .rearrange("p (c b) -> p c b", b=NB)           # [p, c, b]
            vv_sb = cst_sb[:, 6 * H + 2 * NB:] \
                .rearrange("p (c m) -> p c m", m=32)           # [p, c, m] v x32

            # PE warmup while DMAs land: dense N=512 matmuls on a zeroed
            # tile so the HAM clock gate opens before the real stream starts
            warm = cpool.tile([128, S], bf16)
            nc.gpsimd.memset(warm[:], 0.0)
            wpsum = ps_s.tile([64, S], f32, tag="sps")
            for _ in range(10):
                nc.tensor.matmul(wpsum[:], warm[:, :64], warm[:])

            cbias = cpool.tile([128, 2, NB], f32)

            def emit_cmms():
                # c = W_dec @ dec^T  -> [H, NB] f32, kept as tanh bias
                for mc in range(2):
                    pc = ps_s.tile([128, NB], f32, tag="sps")
                    for kc in range(2):
                        nc.tensor.matmul(
                            pc[:],
                            wt_sb[:, 4 + kc, mc * 128:(mc + 1) * 128],
                            de_sb[:, kc, :],
                            start=(kc == 0), stop=(kc == 1),
                        )
                    nc.vector.tensor_copy(cbias[:, mc, :], pc[:])

            def score_mms(sbank, q, u_list):
                # M=32 matmul pair -> score row of one batch replicated
                # across the bank's q-th 32-partition slice
                for mc in range(2):
                    nc.tensor.matmul(sbank[32 * q:32 * (q + 1), :],
                                     vv_sb[:, mc, :], u_list[mc][:],
                                     start=(mc == 0), stop=(mc == 1),
                                     tile_position=(0, 32 * q))

            def group_softmax(g, sbank):
                # softmax for batches 4g..4g+3 straight off the psum bank.
                # no max-subtraction: |score| <= ||v||_1 ~ 10, exp stays
                # comfortably inside f32 range, matching the reference to
                # rounding error.
                prob = smpool.tile([128, S], f32, tag="pr")
                sums = smpool.tile([128, 1], f32, tag="sm")
                nc.scalar.activation(prob[:], sbank[:], AF.Exp,
                                     accum_out=sums[:])
                recip = smpool.tile([128, 1], f32, tag="rc")
                nc.vector.reciprocal(recip[:], sums[:])
                outp = smpool.tile([128, S], f32, tag="op")
                nc.vector.tensor_scalar_mul(outp[:], prob[:], recip[:])
                # early groups ride the idle gpsimd SWDGE so the sync queue
                # keeps feeding input; the last group stays on sync (SWDGE's
                # ~2us completion latency would land in the kernel tail)
                dma_eng = nc.sync if g == NB // 4 - 1 else nc.gpsimd
                dma_eng.dma_start(
                    out=out_ext.ap()[4 * g:4 * (g + 1), :],
                    in_=outp[:].rearrange("(q r) s -> q r s", r=32)[:, 0, :])

            def emit_group_scores(g, items):
                sbank = ps_s.tile([128, S], f32, tag="sps")
                for q, (b, u_list) in enumerate(items):
                    score_mms(sbank, q, u_list)
                group_softmax(g, sbank)

            # ---- main loop over batches ----
            state = {}
            pending = []
            b0 = 0

            # c-matmuls need only consts (which lands first) — run them
            # before batch 0 so they fill the consts->x0 arrival gap
            emit_cmms()

            def sprinkle(n):
                # dependency-free filler matmuls: keep the HAM busy-window
                # alive across early data-wait gaps (cheap if data is ready)
                for _ in range(n):
                    nc.tensor.matmul(wpsum[:, :64], warm[:, :64], warm[:, :64])

            sprinkle(6)

            for gi, gsz in enumerate(GROUPS):
                x_t = ipool.tile([128, gsz, 2, 2, S], bf16, tag="xt")
                nc.sync.dma_start(
                    out=x_t[:],
                    in_=xin_ext.ap()[:, b0 * 4 * S:(b0 + gsz) * 4 * S])

                for j in range(gsz):
                    b = b0 + j
                    psu_list = []
                    for mc in range(2):
                        psu = ps_u.tile([128, S], f32, tag="ups")
                        for kc in range(2):
                            nc.tensor.matmul(
                                psu[:],
                                wt_sb[:, kc, mc * 128:(mc + 1) * 128],
                                x_t[:, j, 0, kc, :],
                                start=(kc == 0), stop=False,
                            )
                        for kc in range(2):
                            nc.tensor.matmul(
                                psu[:],
                                wt_sb[:, 2 + kc, mc * 128:(mc + 1) * 128],
                                x_t[:, j, 1, kc, :],
                                start=False, stop=(kc == 1),
                            )
                        psu_list.append(psu)
                    u_list = []
                    for mc in range(2):
                        u_bf = upool.tile([128, S], bf16, tag="u")
                        nc.scalar.activation(u_bf[:], psu_list[mc][:], AF.Tanh,
                                             bias=cbias[:, mc, b:b + 1])
                        u_list.append(u_bf)
                    if b <= 1:
                        sprinkle(2)
                    # group score matmuls run 2+ batches behind the mains
                    pending.append((b, u_list))
                    if len(pending) >= 5:
                        g = pending[0][0] // 4
                        emit_group_scores(g, pending[:4])
                        pending = pending[4:]
                b0 += gsz
            while pending:
                g = pending[0][0] // 4
                emit_group_scores(g, pending[:4])
                pending = pending[4:]

    nc.compile()
    return nc


def _get_nc():
    if "nc" not in _cache:
        _cache["nc"] = _build()
    return _cache["nc"]


def make_in_maps(static_hidden, dynamic_hidden, decoder_hidden, W, v):
    bf = ml_dtypes.bfloat16
    # W[0] is [H, 3H]; wt[p, kc*H + m] = W[0][m, kc*128 + p]
    wt = W[0].T.astype(bf).reshape(6, 128, H).transpose(1, 0, 2).reshape(128, 6 * H)
    # v replicated 32x: vv[p, c*32 + m] = v[c*128 + p]
    vv = np.repeat(v[0, 0].astype(bf).reshape(2, 128).T[:, :, None], 32,
                   axis=2).reshape(128, 64)

    sh = static_hidden.astype(bf).reshape(N_CORES, NB, 2, 128, S)
    dh = dynamic_hidden.astype(bf).reshape(N_CORES, NB, 2, 128, S)
    # xin[p, b, t, c, s]; h = c*128 + p
    xin_all = np.stack([sh, dh], axis=2).transpose(0, 4, 1, 2, 3, 5) \
        .reshape(N_CORES, 128, NB * 4 * S)

    in_maps = []
    for i in range(N_CORES):
        sl = slice(i * NB, (i + 1) * NB)
        dect = decoder_hidden[sl].T.astype(bf).reshape(2, 128, NB) \
            .transpose(1, 0, 2).reshape(128, 2 * NB)
        consts = np.concatenate([wt, dect, vv], axis=1)
        in_maps.append({
            "xin": np.ascontiguousarray(xin_all[i]),
            "consts": np.ascontiguousarray(consts),
        })
    return in_maps


def kernel(static_hidden, dynamic_hidden, decoder_hidden, W, v):
    from concourse.bass_utils import run_bass_kernel_spmd

    static_hidden = np.asarray(static_hidden, dtype=np.float32)
    dynamic_hidden = np.asarray(dynamic_hidden, dtype=np.float32)
    decoder_hidden = np.asarray(decoder_hidden, dtype=np.float32)
    W = np.asarray(W, dtype=np.float32)
    v = np.asarray(v, dtype=np.float32)
    nc = _get_nc()
    in_maps = make_in_maps(static_hidden, dynamic_hidden, decoder_hidden, W, v)
    res = run_bass_kernel_spmd(nc, in_maps, list(range(N_CORES)))
    out = np.concatenate([res.results[i]["out"] for i in range(N_CORES)], axis=0)
    return out[:, None, :].astype(np.float32)


# revision 78
# speedup vs baseline: 1.0191x; 1.0055x over previous
"""Trainium2 Bass kernel for nn_Attention_23218593202595.

reference:
    hidden = concat([static, dynamic, broadcast(decoder)], axis=1)   # [B, 3H, S]
    u      = tanh(einsum('hk,bks->bhs', W[0], hidden))               # [B, H, S]
    scores = einsum('h,bhs->bs', v[0,0], u)[:, None, :]              # [B, 1, S]
    out    = softmax(scores, axis=2)

B=256, H=256, S=512.  Pure data parallel over 8 NeuronCores: core i owns
batches [32i, 32i+32).  W/v/decoder-projection are tiny and replicated.

Per core, per batch b (q = b%4, one PSUM score bank per 4-batch group):
    psum_u[mc]          = sum_kc Wt[kc, mc]^T @ x[kc]   (x = [static;dynamic], bf16)
    u[mc]               = tanh(psum_u[mc] + c[:, b])    (ScalarE, c = W_dec @ dec)
    sbank[32q:32q+32]  += (v ox 1_32)[mc]^T @ u[mc]     (M=32 matmul -> 32 equal rows,
                                                         pipelined 1 batch behind)
After each 4-batch group the softmax runs directly on the psum bank
(reduce-max -> exp w/ fused sum -> reciprocal -> scale) and rows
{0,32,64,96} are DMAed to the output.

All inputs are converted to bf16 and pre-swizzled on the host into
partition-major layouts so every DMA lands as 128 contiguous runs.
"""
import sys

if "/opt/trn_rl_repo" not in sys.path:
    sys.path.insert(0, "/opt/trn_rl_repo")

import numpy as np
import ml_dtypes

B, H, S = 256, 256, 512
N_CORES = 8
NB = B // N_CORES                    # batches per core
GROUPS = [1, 1] + [2] * 15           # batches per dma group (sum = NB)

_cache = {}


def _build():
    import concourse.bass as bass
    import concourse.mybir as mybir
    import concourse.tile as tile
    from concourse import bacc

    f32 = mybir.dt.float32
    bf16 = mybir.dt.bfloat16
    AF = mybir.ActivationFunctionType

    nc = bacc.Bacc("TRN2", target_bir_lowering=False, debug=False,
                   num_devices=N_CORES)

    # all host-preswizzled, partition-major.  xin interleaves static+dynamic
    # per batch as [b, tensor, c, s] so one DMA covers a whole batch group;
    # consts packs wt | dect | vv into a single transfer.
    CW = 6 * H + 2 * NB + 2 * 32
    xin_ext = nc.declare_dram_parameter("xin", [128, NB * 4 * S], bf16, isOutput=False)
    cst_ext = nc.declare_dram_parameter("consts", [128, CW], bf16, isOutput=False)
    out_ext = nc.declare_dram_parameter("out", [NB, S], f32, isOutput=True)

    with tile.TileContext(nc) as tc:
        with (
            tc.tile_pool(name="const", bufs=1) as cpool,
            tc.tile_pool(name="inp", bufs=4) as ipool,
            tc.tile_pool(name="upool", bufs=17) as upool,
            tc.tile_pool(name="smpool", bufs=2) as smpool,
            tc.tile_pool(name="ps_u", bufs=4, space=bass.MemorySpace.PSUM) as ps_u,
            tc.tile_pool(name="ps_s", bufs=4, space=bass.MemorySpace.PSUM) as ps_s,
        ):
            # ---- constants: one DMA, first on the sync queue ----
            cst_sb = cpool.tile([128, CW], bf16)
            nc.sync.dma_start(out=cst_sb[:], in_=cst_ext.ap())
            wt_sb = cst_sb[:, :6 * H] \
                .rearrange("p (c m) -> p c m", m=H)            # [p, kc, m]
            de_sb = cst_sb[:, 6 * H:6 * H + 2 * NB] \
                .rearrange("p (c b) -> p c b", b=NB)           # [p, c, b]
            vv_sb = cst_sb[:, 6 * H + 2 * NB:] \
                .rearrange("p (c m) -> p c m", m=32)           # [p, c, m] v x32

            # PE warmup while DMAs land: dense N=512 matmuls on a zeroed
            # tile so the HAM clock gate opens before the real stream starts
            warm = cpool.tile([128, S], bf16)
            nc.gpsimd.memset(warm[:], 0.0)
            wpsum = ps_s.tile([64, S], f32, tag="sps")
            for _ in range(10):
                nc.tensor.matmul(wpsum[:], warm[:, :64], warm[:])

            cbias = cpool.tile([128, 2, NB], f32)

            def emit_cmms():
                # c = W_dec @ dec^T  -> [H, NB] f32, kept as tanh bias
                for mc in range(2):
                    pc = ps_s.tile([128, NB], f32, tag="sps")
                    for kc in range(2):
                        nc.tensor.matmul(
                            pc[:],
                            wt_sb[:, 4 + kc, mc * 128:(mc + 1) * 128],
                            de_sb[:, kc, :],
                            start=(kc == 0), stop=(kc == 1),
                        )
                    nc.vector.tensor_copy(cbias[:, mc, :], pc[:])

            def score_mms(sbank, q, u_list):
                # M=32 matmul pair -> score row of one batch replicated
                # across the bank's q-th 32-partition slice
                for mc in range(2):
                    nc.tensor.matmul(sbank[32 * q:32 * (q + 1), :],
                                     vv_sb[:, mc, :], u_list[mc][:],
                                     start=(mc == 0), stop=(mc == 1),
                                     tile_position=(0, 32 * q))

            def group_softmax(g, sbank):
                # softmax for batches 4g..4g+3 straight off the psum bank.
                # no max-subtraction: |score| <= ||v||_1 ~ 10, exp stays
                # comfortably inside f32 range, matching the reference to
                # rounding error.
                prob = smpool.tile([128, S], f32, tag="pr")
                sums = smpool.tile([128, 1], f32, tag="sm")
                nc.scalar.activation(prob[:], sbank[:], AF.Exp,
                                     accum_out=sums[:])
                recip = smpool.tile([128, 1], f32, tag="rc")
                nc.vector.reciprocal(recip[:], sums[:])
                outp = smpool.tile([128, S], f32, tag="op")
                nc.vector.tensor_scalar_mul(outp[:], prob[:], recip[:])
                # early groups ride the idle gpsimd SWDGE so the sync queue
                # keeps feeding input; the last group stays on sync (SWDGE's
                # ~2us completion latency would land in the kernel tail)
                dma_eng = nc.sync if g == NB // 4 - 1 else nc.gpsimd
                dma_eng.dma_start(
                    out=out_ext.ap()[4 * g:4 * (g + 1), :],
                    in_=outp[:].rearrange("(q r) s -> q r s", r=32)[:, 0, :])

            def emit_group_scores(g, items):
                sbank = ps_s.tile([128, S], f32, tag="sps")
                for q, (b, u_list) in enumerate(items):
                    score_mms(sbank, q, u_list)
                group_softmax(g, sbank)

            # ---- main loop over batches ----
            state = {}
            pending = []
            b0 = 0

            # c-matmuls need only consts (which lands first) — run them
            # before batch 0 so they fill the consts->x0 arrival gap
            emit_cmms()

            for gi, gsz in enumerate(GROUPS):
                x_t = ipool.tile([128, gsz, 2, 2, S], bf16, tag="xt")
                nc.sync.dma_start(
                    out=x_t[:],
                    in_=xin_ext.ap()[:, b0 * 4 * S:(b0 + gsz) * 4 * S])

                for j in range(gsz):
                    b = b0 + j
                    psu_list = []
                    for mc in range(2):
                        psu = ps_u.tile([128, S], f32, tag="ups")
                        for kc in range(2):
                            nc.tensor.matmul(
                                psu[:],
                                wt_sb[:, kc, mc * 128:(mc + 1) * 128],
                                x_t[:, j, 0, kc, :],
                                start=(kc == 0), stop=False,
                            )
                        for kc in range(2):
                            nc.tensor.matmul(
                                psu[:],
                                wt_sb[:, 2 + kc, mc * 128:(mc + 1) * 128],
                                x_t[:, j, 1, kc, :],
                                start=False, stop=(kc == 1),
                            )
                        psu_list.append(psu)
                    u_list = []
                    for mc in range(2):
                        u_bf = upool.tile([128, S], bf16, tag="u")
                        nc.scalar.activation(u_bf[:], psu_list[mc][:], AF.Tanh,
                                             bias=cbias[:, mc, b:b + 1])
                        u_list.append(u_bf)
                    # group score matmuls run 2+ batches behind the mains
                    pending.append((b, u_list))
                    if len(pending) >= 5:
                        g = pending[0][0] // 4
                        emit_group_scores(g, pending[:4])
                        pending = pending[4:]
                b0 += gsz
            while pending:
                g = pending[0][0] // 4
                emit_group_scores(g, pending[:4])
                pending = pending[4:]

    nc.compile()
    return nc


def _get_nc():
    if "nc" not in _cache:
        _cache["nc"] = _build()
    return _cache["nc"]


def make_in_maps(static_hidden, dynamic_hidden, decoder_hidden, W, v):
    bf = ml_dtypes.bfloat16
    # W[0] is [H, 3H]; wt[p, kc*H + m] = W[0][m, kc*128 + p]
    wt = W[0].T.astype(bf).reshape(6, 128, H).transpose(1, 0, 2).reshape(128, 6 * H)
    # v replicated 32x: vv[p, c*32 + m] = v[c*128 + p]
    vv = np.repeat(v[0, 0].astype(bf).reshape(2, 128).T[:, :, None], 32,
                   axis=2).reshape(128, 64)

    sh = static_hidden.astype(bf).reshape(N_CORES, NB, 2, 128, S)
    dh = dynamic_hidden.astype(bf).reshape(N_CORES, NB, 2, 128, S)
    # xin[p, b, t, c, s]; h = c*128 + p
    xin_all = np.stack([sh, dh], axis=2).transpose(0, 4, 1, 2, 3, 5) \
        .reshape(N_CORES, 128, NB * 4 * S)

    in_maps = []
    for i in range(N_CORES):
        sl = slice(i * NB, (i + 1) * NB)
        dect = decoder_hidden[sl].T.astype(bf).reshape(2, 128, NB) \
            .transpose(1, 0, 2).reshape(128, 2 * NB)
        consts = np.concatenate([wt, dect, vv], axis=1)
        in_maps.append({
            "xin": np.ascontiguousarray(xin_all[i]),
            "consts": np.ascontiguousarray(consts),
        })
    return in_maps


def kernel(static_hidden, dynamic_hidden, decoder_hidden, W, v):
    from concourse.bass_utils import run_bass_kernel_spmd

    static_hidden = np.asarray(static_hidden, dtype=np.float32)
    dynamic_hidden = np.asarray(dynamic_hidden, dtype=np.float32)
    decoder_hidden = np.asarray(decoder_hidden, dtype=np.float32)
    W = np.asarray(W, dtype=np.float32)
    v = np.asarray(v, dtype=np.float32)
    nc = _get_nc()
    in_maps = make_in_maps(static_hidden, dynamic_hidden, decoder_hidden, W, v)
    res = run_bass_kernel_spmd(nc, in_maps, list(range(N_CORES)))
    out = np.concatenate([res.results[i]["out"] for i in range(N_CORES)], axis=0)
    return out[:, None, :].astype(np.float32)
